# revision 3
# baseline (speedup 1.0000x reference)
"""MoE (noisy top-2-of-8 gating) Trainium2 kernel — pair-cover expert sharding.

Each core hosts a block of 4 experts chosen so that every expert-pair is
covered by some block; the host assigns each token to a core whose block
contains both of its top-2 experts (exactly 1024 tokens/core). Both of a
token's expert rows are therefore computed on one core: no collectives, and
per-core weight traffic is 4 experts (16.8MB) instead of 8 (33.5MB).

Device phases per core (ACT table-set loads minimized to ~5):
  1. gating logits+stddev (exp then ln, batched), 2. fc1+GELU for all 4
  segments (h parked in SBUF), 3. top-2 softmax gates (exp), 4. fc2 + exp
  (PSUM->ACT->bf16) + gather/scale/sum combines pipelined per stored a-tile,
  5. final Ln + y stores.
Segments are ~512 rows so matmuls run at N~512 with amortized weight loads.
Weights are host-packed so per-segment tensors are a few large DMAs, with
w1 split into h-major quarters so fc1 starts as soon as the first lands.
Phase order on each engine queue is pinned with tile_set_cur_wait.
"""

import numpy as np
import ml_dtypes

import concourse.bacc as bacc
import concourse.bass as bass
import concourse.mybir as mybir
import concourse.tile as tile
from concourse.bass_utils import run_bass_kernel_spmd
from concourse.masks import make_identity

BF16 = mybir.dt.bfloat16
FP32 = mybir.dt.float32
AF = mybir.ActivationFunctionType

N, D, H, E, TOPK = 8192, 512, 2048, 8, 2
NC = 8
NS = N // NC          # tokens per core
P = 128
NTT = NS // P         # token tiles per core (8)
DC = D // P           # d chunks (4)
HC = H // P           # hidden chunks (16)
FC = (2 * D) // P     # gate feature chunks (8)
SEG = 4               # experts hosted per core

BLOCKS = [(0, 1, 2, 3), (4, 5, 6, 7), (0, 1, 4, 5), (2, 3, 6, 7),
          (0, 1, 6, 7), (2, 3, 4, 5), (0, 2, 4, 6), (1, 3, 5, 7)]

_nc_cache: dict = {}


def _atiles(caps):
    """Global a-store tile list [(seg, tt, m, pref)] in emission order."""
    offs = np.concatenate([[0], np.cumsum(caps)]).astype(int)
    out = []
    for s in range(SEG):
        cap = int(caps[s])
        ntt = (cap + P - 1) // P
        for tt in range(ntt):
            m = min(P, cap - tt * P)
            out.append((s, tt, m, int(offs[s]) + tt * P + m))
    return out


def _build_nc(caps, rtiles=None, reps=1, timing=False, use_b2=False,
              gelu_sub=False, fused_gather=False, use_b1=False, paired=False):
    caps = tuple(int(c) for c in caps)
    R = sum(caps)
    offs = np.concatenate([[0], np.cumsum(caps)]).astype(int)
    xoff = np.concatenate([[0], np.cumsum([DC * c for c in caps])]).astype(int)
    hoff = np.concatenate([[0], np.cumsum([HC * c for c in caps])]).astype(int)
    atiles = _atiles(caps)
    if rtiles is None:
        rtiles = (len(atiles) - 1,) * NTT
    gelu_af = AF.Tanh if gelu_sub else AF.Gelu

    nc = bacc.Bacc("TRN2", target_bir_lowering=False, debug=False)

    if timing:
        def param(name, shape, dtype):
            return nc.dram_tensor(name, shape, dtype)
        nc.declare_dram_parameter("tdin", [1, 4], FP32, isOutput=False)
        y_d = nc.dram_tensor("y", [NS, D], FP32)
        yo_d = nc.declare_dram_parameter("yo", [1, 4], FP32, isOutput=True)
    else:
        def param(name, shape, dtype):
            return nc.declare_dram_parameter(name, shape, dtype, isOutput=False)
        y_d = nc.declare_dram_parameter("y", [NS, D], FP32, isOutput=True)

    xt_d = param("xt", [P, DC * R], BF16)
    gft_d = param("gft", [P, 2 * FC * 512], BF16)
    nst_d = param("nst", [E, NS], FP32)
    wg_d = param("wg", [P, FC * E], BF16)
    wn_d = param("wn", [P, FC * E], BF16)
    w1_d = param("w1", [SEG, P, HC * DC * P], BF16)   # [p, hc*512 + d*128 + j]
    w2_d = param("w2", [SEG, P, HC * D], BF16)        # [p, hc*D + dd]
    b1_d = param("b1", [P, SEG * HC], FP32)
    b2_d = param("b2", [SEG, D], BF16)
    j12_d = param("j12", [P, 2 * NTT], mybir.dt.int32)
    g3start = next(g for g, (s_, tt_, m_, pref_) in enumerate(atiles)
                   if s_ == SEG - 1)
    perm_tiles = [t for t in range(NTT) if rtiles[t] >= g3start]
    pmt_d = param("pmt", [P, max(1, len(perm_tiles)) * 2 * P], BF16)

    with tile.TileContext(nc) as tc:
        with (
            tc.tile_pool(name="const", bufs=1) as constp,
            tc.tile_pool(name="gate", bufs=1) as gatep,
            tc.tile_pool(name="gfp", bufs=2) as gfp,
            tc.tile_pool(name="w1p", bufs=2) as w1p,
            tc.tile_pool(name="w2p", bufs=2) as w2p,
            tc.tile_pool(name="spool", bufs=2) as sp,
            tc.tile_pool(name="oncep", bufs=1) as onp,
            tc.tile_pool(name="psumg", bufs=1, space="PSUM") as ppg,
            tc.tile_pool(name="psumtl", bufs=1, space="PSUM") as ptl,
            tc.tile_pool(name="psum1", bufs=2, space="PSUM") as pp,
            tc.tile_pool(name="psum2", bufs=2, space="PSUM") as pp2,
            tc.tile_pool(name="dram", bufs=1, space="DRAM") as dp,
        ):
            ident = constp.tile([P, P], FP32)
            make_identity(nc, ident[:])
            ones1 = constp.tile([1, P], BF16)
            nc.vector.memset(ones1[:], 1.0)

            def body(_i=None):
                W = tc.tile_set_cur_wait
                # ---------- PE warm-up (cost-model p-state ramp) + DMAs,
                # emitted in intended arrival order: gft0 first so gating MMs
                # follow the warm-up burst seamlessly ----------
                W(0.00005)
                wzero = onp.tile([P, 512], BF16, tag="wzero")
                nc.vector.memset(wzero[:], 0.0)
                wsink = onp.tile([1, 4], FP32, tag="wsink")
                gf_tiles = []
                gfsb0 = gfp.tile([P, FC * 512], BF16, tag="gf")
                nc.sync.dma_start(out=gfsb0[:], in_=gft_d[:, 0 : FC * 512])
                gf_tiles.append(gfsb0)
                wgsb = gatep.tile([P, FC * E], BF16, tag="wgsb")
                wnsb = gatep.tile([P, FC * E], BF16, tag="wnsb")
                nc.sync.dma_start(out=wgsb[:], in_=wg_d[:])
                nc.sync.dma_start(out=wnsb[:], in_=wn_d[:])
                gfsb1 = gfp.tile([P, FC * 512], BF16, tag="gf")
                nc.sync.dma_start(out=gfsb1[:], in_=gft_d[:, FC * 512 : 2 * FC * 512])
                gf_tiles.append(gfsb1)
                for k in range(16):
                    wps = ppg.tile([P, P], FP32, tag="gate_ps")
                    nc.tensor.matmul(
                        wps[:],
                        lhsT=wzero[:, 0:P],
                        rhs=wzero[:, 0:P],
                        start=True,
                        stop=True,
                    )
                    if k == 15:
                        nc.vector.tensor_copy(wsink[:], wps[0:1, 0:4])
                nc.sync.dma_start(out=y_d[0:1, 0:4], in_=wsink[:])
                W(0.00015)
                w1_tiles = []
                xsb = gatep.tile([P, DC * R], BF16, tag="xsb")
                w1sb0 = w1p.tile([P, HC * DC * P], BF16, tag="w1")
                Q = HC * DC * P // 4
                nc.sync.dma_start(out=w1sb0[:, 0:Q], in_=w1_d[0][:, 0:Q])
                nc.sync.dma_start(
                    out=xsb[:, xoff[0] : xoff[1]], in_=xt_d[:, xoff[0] : xoff[1]]
                )
                w1_tiles.append(w1sb0)
                nssb = gatep.tile([E, NS], FP32, tag="nssb")
                nc.sync.dma_start(out=nssb[:], in_=nst_d[:])
                W(0.0003)
                for q in range(1, 4):
                    nc.sync.dma_start(
                        out=w1sb0[:, q * Q : (q + 1) * Q],
                        in_=w1_d[0][:, q * Q : (q + 1) * Q],
                    )
                b1sb = gatep.tile([P, SEG * HC], FP32, tag="b1sb")
                nc.sync.dma_start(out=b1sb[:], in_=b1_d[:])
                b2sb = gatep.tile([SEG, D], BF16, tag="b2sb")
                if use_b2:
                    nc.sync.dma_start(out=b2sb[:], in_=b2_d[:])
                j12sb = gatep.tile([P, 2 * NTT], mybir.dt.int32, tag="j12sb")
                if timing:
                    nc.vector.memset(j12sb[:], 0)
                else:
                    nc.sync.dma_start(out=j12sb[:], in_=j12_d[:])
                pmsb = gatep.tile([P, max(1, len(perm_tiles)) * 2 * P], BF16,
                                  tag="pmsb")
                if perm_tiles:
                    if timing:
                        nc.vector.memset(pmsb[:], 0)
                    else:
                        nc.sync.dma_start(out=pmsb[:], in_=pmt_d[:])
                W(0.0004)
                for s in range(1, SEG):
                    w1sb = w1p.tile([P, HC * DC * P], BF16, tag="w1")
                    for hf in range(2):
                        nc.sync.dma_start(
                            out=w1sb[:, hf * 2 * Q : (hf + 1) * 2 * Q],
                            in_=w1_d[s][:, hf * 2 * Q : (hf + 1) * 2 * Q],
                        )
                    w1_tiles.append(w1sb)
                    nc.sync.dma_start(
                        out=xsb[:, xoff[s] : xoff[s + 1]],
                        in_=xt_d[:, xoff[s] : xoff[s + 1]],
                    )
                W(0.0005)
                w2_tiles = []
                for s in range(SEG):
                    w2sb = w2p.tile([P, HC * D], BF16, tag="w2")
                    nc.sync.dma_start(out=w2sb[:], in_=w2_d[s])
                    w2_tiles.append(w2sb)

                # ---------- gating: noise+clean logits; exp/ln batched ------
                lg_sb = gatep.tile([E, NS], FP32, tag="lg")
                std_tiles = []
                for t in range(2):
                    W(0.0008 + 0.0007 * t)
                    t0, t1 = t * 512, (t + 1) * 512
                    gfsb = gf_tiles[t]
                    nps = ppg.tile([E, 512], FP32, tag="gate_ps")
                    for c in range(FC):
                        nc.tensor.matmul(
                            nps[:],
                            lhsT=wnsb[:, c * E : (c + 1) * E],
                            rhs=gfsb[:, c * 512 : (c + 1) * 512],
                            start=(c == 0),
                            stop=(c == FC - 1),
                        )
                    std_t = onp.tile([E, 512], FP32, tag=f"std{t}")
                    nc.scalar.activation(std_t[:], nps[:], AF.Exp)
                    nc.vector.tensor_scalar_add(std_t[:], std_t[:], 1.0)
                    std_tiles.append(std_t)
                    W(0.0009 + 0.0007 * t)
                    cps = ppg.tile([E, 512], FP32, tag="gate_ps")
                    for c in range(FC):
                        nc.tensor.matmul(
                            cps[:],
                            lhsT=wgsb[:, c * E : (c + 1) * E],
                            rhs=gfsb[:, c * 512 : (c + 1) * 512],
                            start=(c == 0),
                            stop=(c == FC - 1),
                        )
                    nc.vector.tensor_copy(lg_sb[:, t0:t1], cps[:])
                W(0.003)
                for t in range(2):
                    std = std_tiles[t][:]
                    nc.scalar.activation(std, std, AF.Ln)
                    nc.vector.tensor_scalar_add(std, std, 1e-2)
                    nc.vector.tensor_mul(std, std, nssb[:, t * 512 : (t + 1) * 512])
                    nc.vector.tensor_add(
                        lg_sb[:, t * 512 : (t + 1) * 512],
                        lg_sb[:, t * 512 : (t + 1) * 512],
                        std,
                    )

                # ---------- fc1 + GELU for all segments; h parked in SBUF ---
                hpark = gatep.tile([P, HC * R], BF16, tag="hpark")
                for s in range(SEG):
                    W(0.002 + 0.012 * s)
                    cap = caps[s]
                    w1sb = w1_tiles[s]
                    chunks = [(0, min(cap, 512))]
                    if cap > 512:
                        chunks.append((512, cap))
                    for q in range(HC // 2):
                        if s == 0:
                            W(0.001 if q < 2 else 0.002)
                        for ci, (n0, n1) in enumerate(chunks):
                            ln = n1 - n0
                            if paired:
                                if ci == 0:
                                    pst = pp.tile([P, 2 * 512], FP32, tag="fc1_ps")
                                else:
                                    pst = ptl.tile([P, 2 * 128], FP32, tag="fc1_tl")
                                psts = [pst[:, 0:ln], pst[:, ln : 2 * ln]]
                            else:
                                if ci == 0:
                                    pa = pp.tile([P, 512], FP32, tag="fc1_ps")
                                    pb = pp.tile([P, 512], FP32, tag="fc1_ps")
                                else:
                                    pa = ptl.tile([P, 128], FP32, tag="fc1_tl")
                                    pb = ptl.tile([P, 128], FP32, tag="fc1_tl")
                                psts = [pa[:, 0:ln], pb[:, 0:ln]]
                            for hin in range(2):
                                h = 2 * q + hin
                                for d in range(DC):
                                    nc.tensor.matmul(
                                        psts[hin],
                                        lhsT=w1sb[:, h * DC * P + d * P : h * DC * P + (d + 1) * P],
                                        rhs=xsb[:, xoff[s] + d * cap + n0 : xoff[s] + d * cap + n1],
                                        start=(d == 0),
                                        stop=(d == DC - 1),
                                    )
                            if use_b1 or not paired:
                                for hin in range(2):
                                    h = 2 * q + hin
                                    nc.scalar.activation(
                                        hpark[:, hoff[s] + h * cap + n0 : hoff[s] + h * cap + n1],
                                        psts[hin],
                                        gelu_af,
                                        bias=b1sb[:, s * HC + h : s * HC + h + 1],
                                    )
                            else:
                                outv = hpark[
                                    :, hoff[s] + 2 * q * cap : hoff[s] + (2 * q + 2) * cap
                                ].rearrange("p (k n) -> p k n", k=2)[:, :, n0:n1]
                                inv = pst[:, 0 : 2 * ln].rearrange(
                                    "p (k n) -> p k n", k=2
                                )
                                nc.scalar.activation(outv, inv, gelu_af)

                # ---------- top-2 softmax gates (deferred: its Exp lands in
                # the same table-set region as the fc2 exps) ----------------
                W(0.085)
                g1sb = gatep.tile([P, NTT], FP32, tag="g1")
                g2sb = gatep.tile([P, NTT], FP32, tag="g2")
                trp_t = ppg.tile([P, 512], FP32, tag="gate_ps")
                trp = trp_t[:, 0 : NTT * E]
                for t in range(NTT):
                    nc.tensor.transpose(
                        trp[:, t * E : (t + 1) * E],
                        lg_sb[:, t * P : (t + 1) * P],
                        ident[:E, :E],
                    )
                lt8 = onp.tile([P, NTT * E], FP32, tag="lt8")
                nc.vector.tensor_copy(lt8[:], trp[:])
                mx8 = onp.tile([P, NTT * 8], FP32, tag="mx8")
                d21a = onp.tile([P, NTT], FP32, tag="d21a")
                e21a = onp.tile([P, NTT], FP32, tag="e21a")
                t1ga = onp.tile([P, NTT], FP32, tag="t1ga")
                for t in range(NTT):
                    nc.vector.max(
                        out=mx8[:, t * 8 : (t + 1) * 8],
                        in_=lt8[:, t * E : (t + 1) * E],
                    )
                    nc.vector.tensor_sub(
                        d21a[:, t : t + 1],
                        mx8[:, t * 8 + 1 : t * 8 + 2],
                        mx8[:, t * 8 : t * 8 + 1],
                    )
                nc.scalar.activation(e21a[:], d21a[:], AF.Exp)
                nc.vector.tensor_scalar_add(t1ga[:], e21a[:], 1.0)
                nc.vector.reciprocal(g1sb[:], t1ga[:])
                nc.vector.tensor_mul(g2sb[:], g1sb[:], e21a[:])

                # ---------- fc2 + exp + pipelined combines ----------
                a_dram = dp.tile([R, D], BF16, tag="a_tab")
                comb = gatep.tile([P, NTT * D], FP32, tag="comb")

                def emit_combine(t, pref, perm_from=None):
                    # perm_from = (asb_tile, m, perm_idx): rows of the final
                    # atile come via a PE permutation-matmul from SBUF; the
                    # gather is bounded to earlier rows (OOB rows skipped
                    # into the pre-zeroed buffer)
                    b12 = sp.tile([P, 2 * D], BF16, tag="b12")
                    if perm_from is not None:
                        nc.vector.memset(b12[:], 0.0)
                    if fused_gather:
                        nc.gpsimd.indirect_dma_start(
                            out=b12[:],
                            out_offset=None,
                            in_=a_dram[0:pref, :],
                            in_offset=bass.IndirectOffsetOnAxis(
                                ap=j12sb[:, 2 * t : 2 * t + 2], axis=0
                            ),
                        )
                    else:
                        bc = (pref - 1) if perm_from is not None else None
                        nc.gpsimd.indirect_dma_start(
                            out=b12[:, 0:D],
                            out_offset=None,
                            in_=a_dram[0:pref, :],
                            in_offset=bass.IndirectOffsetOnAxis(
                                ap=j12sb[:, 2 * t : 2 * t + 1], axis=0
                            ),
                            bounds_check=bc,
                            oob_is_err=(bc is None),
                        )
                        nc.gpsimd.indirect_dma_start(
                            out=b12[:, D : 2 * D],
                            out_offset=None,
                            in_=a_dram[0:pref, :],
                            in_offset=bass.IndirectOffsetOnAxis(
                                ap=j12sb[:, 2 * t + 1 : 2 * t + 2], axis=0
                            ),
                            bounds_check=bc,
                            oob_is_err=(bc is None),
                        )
                    AL = mybir.AluOpType
                    ct = comb[:, t * D : (t + 1) * D]
                    # gathered part first (off the late critical chain):
                    # ct = g1*b12_1 + g2*b12_2 in two fused DVE ops
                    nc.vector.tensor_scalar_mul(ct, b12[:, 0:D],
                                                g1sb[:, t : t + 1])
                    nc.vector.scalar_tensor_tensor(
                        ct, b12[:, D : 2 * D], g2sb[:, t : t + 1], ct,
                        AL.mult, AL.add,
                    )
                    if perm_from is not None:
                        pasb, pm, pidx = perm_from
                        ppa = pp2.tile([P, D], FP32, tag="perm_ps")
                        ppb = pp2.tile([P, D], FP32, tag="perm_ps")
                        for k, ppk in enumerate((ppa, ppb)):
                            nc.tensor.matmul(
                                ppk[:],
                                lhsT=pmsb[0:pm, (2 * pidx + k) * P : (2 * pidx + k + 1) * P],
                                rhs=pasb[0:pm, :],
                                start=True,
                                stop=True,
                            )
                        nc.vector.scalar_tensor_tensor(
                            ct, ppa[:], g1sb[:, t : t + 1], ct,
                            AL.mult, AL.add,
                        )
                        nc.vector.scalar_tensor_tensor(
                            ct, ppb[:], g2sb[:, t : t + 1], ct,
                            AL.mult, AL.add,
                        )

                pending_perm = []
                for g, (s, tt, m, pref) in enumerate(atiles):
                    W(0.09 + 0.003 * g)
                    flush_perm = pending_perm
                    pending_perm = []
                    cap = caps[s]
                    w2sb = w2_tiles[s]
                    ps2 = pp2.tile([P, D], FP32, tag="fc2_ps")
                    for h in range(HC):
                        nc.tensor.matmul(
                            ps2[:m],
                            lhsT=hpark[:, hoff[s] + h * cap + tt * P : hoff[s] + h * cap + tt * P + m],
                            rhs=w2sb[:, h * D : (h + 1) * D],
                            start=(h == 0),
                            stop=(h == HC - 1 and not use_b2),
                        )
                    if use_b2:
                        nc.tensor.matmul(
                            ps2[:m],
                            lhsT=ones1[:, :m],
                            rhs=b2sb[s : s + 1, :],
                            start=False,
                            stop=True,
                        )
                    asb = sp.tile([P, D], BF16, tag="a_sb")
                    nc.scalar.activation(asb[:m], ps2[:m], AF.Exp)
                    # the store is only needed if some gather reads this atile:
                    # unpermuted combines read atiles <= their rtile; permuted
                    # ones read atiles < their rtile
                    store_needed = any(
                        (rtiles[t] >= g and t not in perm_tiles)
                        or (rtiles[t] > g and t in perm_tiles)
                        for t in range(NTT)
                    )
                    if store_needed:
                        nc.sync.dma_start(
                            out=a_dram[offs[s] + tt * P : offs[s] + tt * P + m, :],
                            in_=asb[:m],
                        )
                    for t in range(NTT):
                        if rtiles[t] == g:
                            if t in perm_tiles:
                                pidx = perm_tiles.index(t)
                                prev_pref = atiles[g - 1][3] if g > 0 else 0
                                pending_perm.append(
                                    (t, prev_pref, (asb, m, pidx))
                                )
                            else:
                                emit_combine(t, pref)
                    for (pt, ppref, pfrom) in flush_perm:
                        emit_combine(pt, ppref, perm_from=pfrom)

                # ---------- final Ln (in place) + y stores ----------
                W(0.15)
                for (pt, ppref, pfrom) in pending_perm:
                    emit_combine(pt, ppref, perm_from=pfrom)
                W(0.3)
                for t in range(NTT):
                    nc.scalar.activation(
                        comb[:, t * D : (t + 1) * D],
                        comb[:, t * D : (t + 1) * D],
                        AF.Ln,
                    )
                    nc.sync.dma_start(
                        out=y_d[t * P : (t + 1) * P, :],
                        in_=comb[:, t * D : (t + 1) * D],
                    )
                tc.tile_set_cur_wait(0, enable=False)

            if reps > 1:
                with tc.For_i(0, reps, 1):
                    body()
            else:
                body()
            if timing:
                nc.sync.dma_start(out=yo_d[:], in_=ident[:1, :4])

    nc.compile()
    return nc


def _route(gate_feat, noise, w_gate, w_noise):
    """Host-side routing structure (fp32 numpy, matches jax top-k selection)."""
    clean = gate_feat @ w_gate
    stddev = np.logaddexp(gate_feat @ w_noise, 0.0) + np.float32(1e-2)
    logits = clean.astype(np.float32) + noise * stddev.astype(np.float32)
    top2 = np.argsort(-logits, axis=1, kind="stable")[:, :TOPK].astype(np.int32)
    return top2


def _pair_cands(pid_vals):
    bsets = [frozenset(b) for b in BLOCKS]
    cands = {}
    for p in pid_vals:
        i, j = int(p) // E, int(p) % E
        cs = [c for c, b in enumerate(bsets) if i in b and j in b]
        assert cs, f"pair ({i},{j}) not covered by block design"
        cands[int(p)] = cs
    return cands


def _assign_lp(pair, pid):
    """Pair-level LP: minimize total per-(core,expert) overflow beyond 512
    (ragged fc2 tiles), subject to exact pair counts and NS tokens/core."""
    from scipy.optimize import linprog

    upids = [int(p) for p in np.unique(pid)]
    n_p = {p: int((pid == p).sum()) for p in upids}
    cands = _pair_cands(upids)
    xv = [(p, c) for p in upids for c in cands[p]]
    xi = {k: i for i, k in enumerate(xv)}
    ov = [(c, e) for c in range(NC) for e in BLOCKS[c]]
    oi = {k: len(xv) + i for i, k in enumerate(ov)}
    nvar = len(xv) + len(ov)

    Aeq, beq = [], []
    for p in upids:
        row = np.zeros(nvar)
        for c in cands[p]:
            row[xi[(p, c)]] = 1
        Aeq.append(row)
        beq.append(n_p[p])
    for c in range(NC):
        row = np.zeros(nvar)
        for p in upids:
            if c in cands[p]:
                row[xi[(p, c)]] = 1
        Aeq.append(row)
        beq.append(NS)
    Aub, bub = [], []
    for (c, e) in ov:
        row = np.zeros(nvar)
        for p in upids:
            if c in cands[p] and (p // E == e or p % E == e):
                row[xi[(p, c)]] = 1
        row[oi[(c, e)]] = -1
        Aub.append(row)
        bub.append(512)
    cvec = np.zeros(nvar)
    cvec[len(xv):] = 1.0
    res = linprog(cvec, A_ub=np.array(Aub), b_ub=np.array(bub),
                  A_eq=np.array(Aeq), b_eq=np.array(beq),
                  bounds=[(0, None)] * nvar, method="highs")
    if res.status != 0:
        return None
    x = res.x[: len(xv)]
    xf = {k: int(np.floor(x[i] + 1e-9)) for k, i in xi.items()}
    for p in upids:
        rem = n_p[p] - sum(xf[(p, c)] for c in cands[p])
        fr = sorted(cands[p], key=lambda c: -(x[xi[(p, c)]] - xf[(p, c)]))
        for c in fr[:rem]:
            xf[(p, c)] += 1
    load = np.zeros(NC, dtype=np.int64)
    for (p, c), v in xf.items():
        load[c] += v
    # repair core totals (rounding drift) by shifting along shared pairs
    for _ in range(1000):
        if (load == NS).all():
            break
        moved = False
        for p in upids:
            cs = cands[p]
            for a in cs:
                if load[a] > NS and xf[(p, a)] > 0:
                    for b in cs:
                        if load[b] < NS:
                            xf[(p, a)] -= 1
                            xf[(p, b)] += 1
                            load[a] -= 1
                            load[b] += 1
                            moved = True
                            break
                if moved:
                    break
            if moved:
                break
        if not moved:
            return None
    if not (load == NS).all():
        return None
    return xf


def _assign_greedy(pair, pid):
    """Fallback greedy (load-balanced, per-expert max minimized)."""
    cands = _pair_cands([int(p) for p in np.unique(pid)])
    ncand = np.array([len(cands[int(p)]) for p in pid])
    prio = np.where(ncand == 1, 0, np.where(ncand == 3, 1, 2))
    order = np.argsort(prio, kind="stable")
    core_of_tok = np.full(N, -1, dtype=np.int64)
    load = np.zeros(NC, dtype=np.int64)
    eload = np.zeros((NC, E), dtype=np.int64)
    for t in order:
        cs = cands[int(pid[t])]
        i, j = int(pair[t, 0]), int(pair[t, 1])
        avail = [c for c in cs if load[c] < NS]
        if not avail:
            moved = False
            for c in cs:
                for u in np.where(core_of_tok == c)[0]:
                    for c2 in cands[int(pid[u])]:
                        if c2 != c and load[c2] < NS:
                            core_of_tok[u] = c2
                            load[c] -= 1
                            load[c2] += 1
                            ui, uj = int(pair[u, 0]), int(pair[u, 1])
                            eload[c, ui] -= 1
                            eload[c, uj] -= 1
                            eload[c2, ui] += 1
                            eload[c2, uj] += 1
                            moved = True
                            break
                    if moved:
                        break
                if moved:
                    avail = [c]
                    break
            assert moved, "assignment infeasible"
        best = min(avail, key=lambda c: (load[c], max(eload[c, i], eload[c, j])))
        core_of_tok[t] = best
        load[best] += 1
        eload[best, i] += 1
        eload[best, j] += 1
    assert (load == NS).all()
    return core_of_tok, eload


def _assign_cores(top2):
    """Token -> core assignment: each token to a core whose 4-expert block
    contains both of its experts; exactly NS tokens per core; per-(core,
    expert) loads pushed toward <=512 to minimize ragged fc2 tiles."""
    pair = np.sort(top2, axis=1)
    pid = pair[:, 0] * E + pair[:, 1]
    xf = None
    try:
        xf = _assign_lp(pair, pid)
    except Exception:
        xf = None
    if xf is None:
        return _assign_greedy(pair, pid)
    core_of_tok = np.full(N, -1, dtype=np.int64)
    eload = np.zeros((NC, E), dtype=np.int64)
    for p in np.unique(pid):
        toks = np.where(pid == p)[0]
        pos = 0
        i, j = int(p) // E, int(p) % E
        for c in _pair_cands([int(p)])[int(p)]:
            v = xf.get((int(p), c), 0)
            for t in toks[pos : pos + v]:
                core_of_tok[t] = c
            eload[c, i] += v
            eload[c, j] += v
            pos += v
        assert pos == len(toks)
    assert (core_of_tok >= 0).all()
    return core_of_tok, eload


def _prepare(x, gate_feat, noise, w_gate, w_noise, fc1_w, fc1_b, fc2_w, fc2_b):
    x = np.ascontiguousarray(x, dtype=np.float32)
    gate_feat = np.ascontiguousarray(gate_feat, dtype=np.float32)
    noise = np.ascontiguousarray(noise, dtype=np.float32)
    bf = ml_dtypes.bfloat16

    top2 = _route(gate_feat, noise, w_gate, w_noise)
    core_of_tok, eload = _assign_cores(top2)
    pair = np.sort(top2, axis=1)

    # segment order per core: hosted experts by descending count
    seg_expert = np.zeros((NC, SEG), dtype=np.int64)
    for c in range(NC):
        hosted = np.array(BLOCKS[c])
        cnts = eload[c, hosted]
        seg_expert[c] = hosted[np.argsort(-cnts, kind="stable")]
    seg_cnts = np.stack(
        [[eload[c, seg_expert[c, s]] for s in range(SEG)] for c in range(NC)]
    )
    caps = seg_cnts.max(axis=0)
    offs = np.concatenate([[0], np.cumsum(caps)]).astype(np.int64)
    R = int(offs[-1])

    # a-store tile index of each global row
    atiles = _atiles(caps)
    atile_of_row = np.zeros(R, dtype=np.int64)
    for g, (s, tt, m, pref) in enumerate(atiles):
        r0 = int(offs[s]) + tt * P
        atile_of_row[r0 : r0 + m] = g

    wg_bf = np.ascontiguousarray(w_gate).astype(bf)
    wn_bf = np.ascontiguousarray(w_noise).astype(bf)
    wgp = np.ascontiguousarray(
        wg_bf.reshape(FC, P, E).transpose(1, 0, 2).reshape(P, FC * E)
    )
    wnp = np.ascontiguousarray(
        wn_bf.reshape(FC, P, E).transpose(1, 0, 2).reshape(P, FC * E)
    )

    in_maps = []
    gidx_cores = []
    rtiles_cores = []
    jp_cores = []
    for c in range(NC):
        toks = np.where(core_of_tok == c)[0]        # 1024 global ids, ascending
        t2 = pair[toks]                              # sorted pairs
        seg_of = np.full(E, -1, dtype=np.int64)
        for s in range(SEG):
            seg_of[seg_expert[c, s]] = s

        row_of = {}                                  # (local_t, expert) -> row
        cols = np.zeros(R, dtype=np.int64)           # row -> local token
        for s in range(SEG):
            e = seg_expert[c, s]
            lt = np.where((t2[:, 0] == e) | (t2[:, 1] == e))[0]
            for i, l in enumerate(lt):
                row_of[(int(l), int(e))] = int(offs[s]) + i
            cols[offs[s] : offs[s] + len(lt)] = lt

        e1 = top2[toks, 0]
        e2 = top2[toks, 1]
        j1 = np.array([row_of[(l, int(e1[l]))] for l in range(NS)], dtype=np.int32)
        j2 = np.array([row_of[(l, int(e2[l]))] for l in range(NS)], dtype=np.int32)
        ready = np.maximum(atile_of_row[j1], atile_of_row[j2])
        perm = np.argsort(ready, kind="stable")
        rtiles_cores.append(ready[perm].reshape(NTT, P).max(axis=1))

        # xt [P, DC*R] segment-major: [p, xoff[s] + d*cap_s + i]
        x_loc = x[toks]                              # [NS, D]
        xt = np.zeros((P, DC * R), dtype=bf)
        for s in range(SEG):
            cap = int(caps[s])
            cnt = int(seg_cnts[c, s])
            xs = x_loc[cols[offs[s] : offs[s] + cnt]]         # [cnt, D]
            blk = xs.reshape(cnt, DC, P).transpose(2, 1, 0)   # [P, DC, cnt]
            base = int(4 * offs[s])
            for d in range(DC):
                xt[:, base + d * cap : base + d * cap + cnt] = blk[:, d, :]

        gf_loc = gate_feat[toks][perm]               # [NS, 2D]
        gftp = np.ascontiguousarray(
            gf_loc.reshape(2, 512, FC, P).transpose(3, 0, 2, 1).reshape(P, 2 * FC * 512)
        ).astype(bf)
        ns_loc = noise[toks][perm]

        w1p_ = np.zeros((SEG, P, HC * DC * P), dtype=bf)
        w2p_ = np.zeros((SEG, P, HC * D), dtype=bf)
        b1p_ = np.zeros((P, SEG * HC), dtype=np.float32)
        b2p_ = np.zeros((SEG, D), dtype=bf)
        for s in range(SEG):
            e = int(seg_expert[c, s])
            # w1: [p, hc*512 + d*128 + j] = fc1_w[e, hc*128+j, d*128+p]
            w1p_[s] = (
                np.asarray(fc1_w[e]).reshape(HC, P, DC, P)
                .transpose(3, 0, 2, 1).reshape(P, HC * DC * P).astype(bf)
            )
            # w2: [p, hc*D + dd] = fc2_w[e, dd, hc*128+p]
            w2p_[s] = (
                np.asarray(fc2_w[e]).T.reshape(HC, P, D)
                .transpose(1, 0, 2).reshape(P, HC * D).astype(bf)
            )
            b1p_[:, s * HC : (s + 1) * HC] = np.asarray(fc1_b[e]).reshape(HC, P).T
            b2p_[s] = np.asarray(fc2_b[e]).astype(bf)

        # permutation blocks for combine tiles resolved via PE permute
        atl = _atiles(caps)
        g3start = next(g for g, (s_, tt_, m_, pref_) in enumerate(atl)
                       if s_ == SEG - 1)
        # NOTE: perm_tiles must match the device build (computed from the
        # global rtiles); filled in after the cross-core max — see below.
        in_maps.append({
            "xt": np.ascontiguousarray(xt),
            "gft": gftp,
            "nst": np.ascontiguousarray(ns_loc.T.astype(np.float32)),
            "wg": wgp,
            "wn": wnp,
            "w1": np.ascontiguousarray(w1p_),
            "w2": np.ascontiguousarray(w2p_),
            "b1": b1p_,
            "b2": b2p_,
            "j12": np.ascontiguousarray(
                np.stack([j1[perm].reshape(NTT, P).T,
                          j2[perm].reshape(NTT, P).T], axis=2).reshape(P, 2 * NTT)
            ),
        })
        gidx_cores.append(toks[perm])
        jp = np.stack([j1[perm].reshape(NTT, P).T,
                       j2[perm].reshape(NTT, P).T], axis=2)  # [P, NTT, 2]
        jp_cores.append(jp)

    rtiles = tuple(int(v) for v in np.max(np.stack(rtiles_cores), axis=0))

    # permutation blocks: tiles whose (global) rtile falls in the last
    # segment take their final-atile rows via a PE permute from SBUF
    atl = _atiles(caps)
    g3start = next(g for g, (s_, tt_, m_, pref_) in enumerate(atl)
                   if s_ == SEG - 1)
    perm_tiles = [t for t in range(NTT) if rtiles[t] >= g3start]
    row_base = {g: (int(np.concatenate([[0], np.cumsum(caps)])[s_]) + tt_ * P)
                for g, (s_, tt_, m_, pref_) in enumerate(atl)}
    atile_of = atile_of_row
    for c in range(NC):
        pmt = np.zeros((P, max(1, len(perm_tiles)) * 2 * P), dtype=ml_dtypes.bfloat16)
        jp = jp_cores[c]
        for pidx, t in enumerate(perm_tiles):
            g = rtiles[t]
            base = row_base[g]
            for k in range(2):
                rows = jp[:, t, k]          # [P] row of token (t, p), side k
                inat = atile_of[rows] == g
                for p in np.where(inat)[0]:
                    pmt[rows[p] - base, (2 * pidx + k) * P + p] = 1
        in_maps[c]["pmt"] = np.ascontiguousarray(pmt)
    return tuple(int(v) for v in caps), rtiles, gidx_cores, in_maps


def kernel(x, gate_feat, noise, w_gate, w_noise, fc1_w, fc1_b, fc2_w, fc2_b,
           _reps=1):
    caps, rtiles, gidx_cores, in_maps = _prepare(
        x, gate_feat, noise, w_gate, w_noise, fc1_w, fc1_b, fc2_w, fc2_b
    )
    use_b2 = bool(np.any(np.asarray(fc2_b)))
    use_b1 = bool(np.any(np.asarray(fc1_b)))
    key = (caps, rtiles, int(_reps), use_b2, use_b1)
    if key not in _nc_cache:
        _nc_cache[key] = _build_nc(caps, rtiles, reps=_reps, use_b2=use_b2,
                                   use_b1=use_b1)
    nc = _nc_cache[key]
    try:
        res = run_bass_kernel_spmd(nc, in_maps, core_ids=list(range(NC)))
    except Exception:
        res = run_bass_kernel_spmd(nc, in_maps, core_ids=list(range(NC)))
    y = np.empty((N, D), np.float32)
    for c in range(NC):
        y[gidx_cores[c]] = res.results[c]["y"]
    return y


# revision 5
# speedup vs baseline: 1.0924x; 1.0924x over previous
"""MoE (noisy top-2-of-8 gating) Trainium2 kernel — pair-cover expert sharding.

Each core hosts a block of 4 experts chosen so that every expert-pair is
covered by some block; the host assigns each token to a core whose block
contains both of its top-2 experts (exactly 1024 tokens/core). Both of a
token's expert rows are therefore computed on one core: no collectives, and
per-core weight traffic is 4 experts (16.8MB) instead of 8 (33.5MB).

Device phases per core (ACT table-set loads minimized to ~5):
  1. gating logits+stddev (exp then ln, batched), 2. fc1+GELU for all 4
  segments (h parked in SBUF), 3. top-2 softmax gates (exp), 4. fc2 + exp
  (PSUM->ACT->bf16) + gather/scale/sum combines pipelined per stored a-tile,
  5. final Ln + y stores.
Segments are ~512 rows so matmuls run at N~512 with amortized weight loads.
Weights are host-packed so per-segment tensors are a few large DMAs, with
w1 split into h-major quarters so fc1 starts as soon as the first lands.
Phase order on each engine queue is pinned with tile_set_cur_wait.
"""

import numpy as np
import ml_dtypes

import concourse.bacc as bacc
import concourse.bass as bass
import concourse.mybir as mybir
import concourse.tile as tile
from concourse.bass_utils import run_bass_kernel_spmd
from concourse.masks import make_identity

BF16 = mybir.dt.bfloat16
FP32 = mybir.dt.float32
AF = mybir.ActivationFunctionType

N, D, H, E, TOPK = 8192, 512, 2048, 8, 2
NC = 8
NS = N // NC          # tokens per core
P = 128
NTT = NS // P         # token tiles per core (8)
DC = D // P           # d chunks (4)
HC = H // P           # hidden chunks (16)
FC = (2 * D) // P     # gate feature chunks (8)
SEG = 4               # experts hosted per core

BLOCKS = [(0, 1, 2, 3), (4, 5, 6, 7), (0, 1, 4, 5), (2, 3, 6, 7),
          (0, 1, 6, 7), (2, 3, 4, 5), (0, 2, 4, 6), (1, 3, 5, 7)]

_nc_cache: dict = {}


def _atiles(caps):
    """Global a-store tile list [(seg, tt, m, pref)] in emission order."""
    offs = np.concatenate([[0], np.cumsum(caps)]).astype(int)
    out = []
    for s in range(SEG):
        cap = int(caps[s])
        ntt = (cap + P - 1) // P
        for tt in range(ntt):
            m = min(P, cap - tt * P)
            out.append((s, tt, m, int(offs[s]) + tt * P + m))
    return out


def _build_nc(caps, rtiles=None, reps=1, timing=False, use_b2=False,
              gelu_sub=False, fused_gather=False, use_b1=False, paired=False):
    caps = tuple(int(c) for c in caps)
    R = sum(caps)
    offs = np.concatenate([[0], np.cumsum(caps)]).astype(int)
    xoff = np.concatenate([[0], np.cumsum([DC * c for c in caps])]).astype(int)
    hoff = np.concatenate([[0], np.cumsum([HC * c for c in caps])]).astype(int)
    atiles = _atiles(caps)
    if rtiles is None:
        rtiles = (len(atiles) - 1,) * NTT
    gelu_af = AF.Tanh if gelu_sub else AF.Gelu

    nc = bacc.Bacc("TRN2", target_bir_lowering=False, debug=False)

    if timing:
        def param(name, shape, dtype):
            return nc.dram_tensor(name, shape, dtype)
        nc.declare_dram_parameter("tdin", [1, 4], FP32, isOutput=False)
        y_d = nc.dram_tensor("y", [NS, D], FP32)
        yo_d = nc.declare_dram_parameter("yo", [1, 4], FP32, isOutput=True)
    else:
        def param(name, shape, dtype):
            return nc.declare_dram_parameter(name, shape, dtype, isOutput=False)
        y_d = nc.declare_dram_parameter("y", [NS, D], FP32, isOutput=True)

    xt_d = param("xt", [P, DC * R], BF16)
    gft_d = param("gft", [P, 2 * FC * 512], BF16)
    nst_d = param("nst", [E, NS], FP32)
    wg_d = param("wg", [P, FC * E], BF16)
    wn_d = param("wn", [P, FC * E], BF16)
    w1_d = param("w1", [SEG, P, HC * DC * P], BF16)   # [p, hc*512 + d*128 + j]
    w2_d = param("w2", [SEG, P, HC * D], BF16)        # [p, hc*D + dd]
    b1_d = param("b1", [P, SEG * HC], FP32)
    b2_d = param("b2", [SEG, D], BF16)
    j12_d = param("j12", [P, 2 * NTT], mybir.dt.int32)
    g3start = next(g for g, (s_, tt_, m_, pref_) in enumerate(atiles)
                   if s_ == SEG - 1)
    perm_tiles = [t for t in range(NTT) if rtiles[t] >= g3start]
    pmt_d = param("pmt", [P, max(1, len(perm_tiles)) * 2 * P], BF16)

    with tile.TileContext(nc) as tc:
        with (
            tc.tile_pool(name="const", bufs=1) as constp,
            tc.tile_pool(name="gate", bufs=1) as gatep,
            tc.tile_pool(name="gfp", bufs=2) as gfp,
            tc.tile_pool(name="w1p", bufs=2) as w1p,
            tc.tile_pool(name="w2p", bufs=2) as w2p,
            tc.tile_pool(name="spool", bufs=2) as sp,
            tc.tile_pool(name="oncep", bufs=1) as onp,
            tc.tile_pool(name="psumg", bufs=1, space="PSUM") as ppg,
            tc.tile_pool(name="psumtl", bufs=1, space="PSUM") as ptl,
            tc.tile_pool(name="psum1", bufs=2, space="PSUM") as pp,
            tc.tile_pool(name="psum2", bufs=2, space="PSUM") as pp2,
            tc.tile_pool(name="dram", bufs=1, space="DRAM") as dp,
        ):
            ident = constp.tile([P, P], FP32)
            make_identity(nc, ident[:])
            ones1 = constp.tile([1, P], BF16)
            nc.vector.memset(ones1[:], 1.0)

            def body(_i=None):
                W = tc.tile_set_cur_wait
                # ---------- PE warm-up (cost-model p-state ramp) + DMAs,
                # emitted in intended arrival order: gft0 first so gating MMs
                # follow the warm-up burst seamlessly ----------
                W(0.00005)
                wzero = onp.tile([P, P], BF16, tag="wzero")
                nc.vector.memset(wzero[:], 0.0)
                wsink = onp.tile([1, 4], FP32, tag="wsink")
                gf_tiles = []
                gfsb0 = gfp.tile([P, FC * 512], BF16, tag="gf")
                nc.sync.dma_start(out=gfsb0[:], in_=gft_d[:, 0 : FC * 512])
                gf_tiles.append(gfsb0)
                wgsb = gatep.tile([P, FC * E], BF16, tag="wgsb")
                wnsb = gatep.tile([P, FC * E], BF16, tag="wnsb")
                nc.sync.dma_start(out=wgsb[:], in_=wg_d[:])
                nc.sync.dma_start(out=wnsb[:], in_=wn_d[:])
                gfsb1 = gfp.tile([P, FC * 512], BF16, tag="gf")
                nc.sync.dma_start(out=gfsb1[:], in_=gft_d[:, FC * 512 : 2 * FC * 512])
                gf_tiles.append(gfsb1)
                for k in range(16):
                    wps = ppg.tile([P, P], FP32, tag="gate_ps")
                    nc.tensor.matmul(
                        wps[:],
                        lhsT=wzero[:],
                        rhs=wzero[:],
                        start=True,
                        stop=True,
                    )
                    if k == 15:
                        nc.vector.tensor_copy(wsink[:], wps[0:1, 0:4])
                nc.sync.dma_start(out=y_d[0:1, 0:4], in_=wsink[:])
                W(0.00015)
                w1_tiles = []
                xsb = gatep.tile([P, DC * R], BF16, tag="xsb")
                w1sb0 = w1p.tile([P, HC * DC * P], BF16, tag="w1")
                Q = HC * DC * P // 4
                nc.sync.dma_start(out=w1sb0[:, 0:Q], in_=w1_d[0][:, 0:Q])
                nc.sync.dma_start(
                    out=xsb[:, xoff[0] : xoff[1]], in_=xt_d[:, xoff[0] : xoff[1]]
                )
                w1_tiles.append(w1sb0)
                nssb = gatep.tile([E, NS], FP32, tag="nssb")
                nc.sync.dma_start(out=nssb[:], in_=nst_d[:])
                W(0.0003)
                for q in range(1, 4):
                    nc.sync.dma_start(
                        out=w1sb0[:, q * Q : (q + 1) * Q],
                        in_=w1_d[0][:, q * Q : (q + 1) * Q],
                    )
                b1sb = gatep.tile([P, SEG * HC], FP32, tag="b1sb")
                nc.sync.dma_start(out=b1sb[:], in_=b1_d[:])
                b2sb = gatep.tile([SEG, D], BF16, tag="b2sb")
                if use_b2:
                    nc.sync.dma_start(out=b2sb[:], in_=b2_d[:])
                j12sb = gatep.tile([P, 2 * NTT], mybir.dt.int32, tag="j12sb")
                if timing:
                    nc.vector.memset(j12sb[:], 0)
                else:
                    nc.sync.dma_start(out=j12sb[:], in_=j12_d[:])
                pmsb = gatep.tile([P, max(1, len(perm_tiles)) * 2 * P], BF16,
                                  tag="pmsb")
                if perm_tiles:
                    if timing:
                        nc.vector.memset(pmsb[:], 0)
                    else:
                        nc.sync.dma_start(out=pmsb[:], in_=pmt_d[:])
                W(0.0004)
                for s in range(1, SEG):
                    w1sb = w1p.tile([P, HC * DC * P], BF16, tag="w1")
                    for hf in range(2):
                        nc.sync.dma_start(
                            out=w1sb[:, hf * 2 * Q : (hf + 1) * 2 * Q],
                            in_=w1_d[s][:, hf * 2 * Q : (hf + 1) * 2 * Q],
                        )
                    w1_tiles.append(w1sb)
                    nc.sync.dma_start(
                        out=xsb[:, xoff[s] : xoff[s + 1]],
                        in_=xt_d[:, xoff[s] : xoff[s + 1]],
                    )
                W(0.0005)
                w2_tiles = []
                for s in range(SEG):
                    w2sb = w2p.tile([P, HC * D], BF16, tag="w2")
                    nc.sync.dma_start(out=w2sb[:], in_=w2_d[s])
                    w2_tiles.append(w2sb)

                # ---------- gating: noise+clean logits; exp/ln batched ------
                lg_sb = gatep.tile([E, NS], FP32, tag="lg")
                std_tiles = []
                for t in range(2):
                    W(0.0008 + 0.0007 * t)
                    t0, t1 = t * 512, (t + 1) * 512
                    gfsb = gf_tiles[t]
                    nps = ppg.tile([E, 512], FP32, tag="gate_ps")
                    for c in range(FC):
                        nc.tensor.matmul(
                            nps[:],
                            lhsT=wnsb[:, c * E : (c + 1) * E],
                            rhs=gfsb[:, c * 512 : (c + 1) * 512],
                            start=(c == 0),
                            stop=(c == FC - 1),
                        )
                    std_t = onp.tile([E, 512], FP32, tag=f"std{t}")
                    nc.scalar.activation(std_t[:], nps[:], AF.Exp)
                    nc.vector.tensor_scalar_add(std_t[:], std_t[:], 1.0)
                    std_tiles.append(std_t)
                    W(0.0009 + 0.0007 * t)
                    cps = ppg.tile([E, 512], FP32, tag="gate_ps")
                    for c in range(FC):
                        nc.tensor.matmul(
                            cps[:],
                            lhsT=wgsb[:, c * E : (c + 1) * E],
                            rhs=gfsb[:, c * 512 : (c + 1) * 512],
                            start=(c == 0),
                            stop=(c == FC - 1),
                        )
                    nc.vector.tensor_copy(lg_sb[:, t0:t1], cps[:])
                W(0.003)
                for t in range(2):
                    std = std_tiles[t][:]
                    nc.scalar.activation(std, std, AF.Ln)
                    nc.vector.tensor_scalar_add(std, std, 1e-2)
                    nc.vector.tensor_mul(std, std, nssb[:, t * 512 : (t + 1) * 512])
                    nc.vector.tensor_add(
                        lg_sb[:, t * 512 : (t + 1) * 512],
                        lg_sb[:, t * 512 : (t + 1) * 512],
                        std,
                    )

                # ---------- fc1 + GELU for all segments; h parked in SBUF ---
                hpark = gatep.tile([P, HC * R], BF16, tag="hpark")
                for s in range(SEG):
                    W(0.002 + 0.012 * s)
                    cap = caps[s]
                    w1sb = w1_tiles[s]
                    chunks = [(0, min(cap, 512))]
                    if cap > 512:
                        chunks.append((512, cap))
                    for q in range(HC // 2):
                        if s == 0:
                            W(0.001 if q < 2 else 0.002)
                        for ci, (n0, n1) in enumerate(chunks):
                            ln = n1 - n0
                            if paired:
                                if ci == 0:
                                    pst = pp.tile([P, 2 * 512], FP32, tag="fc1_ps")
                                else:
                                    pst = ptl.tile([P, 2 * 128], FP32, tag="fc1_tl")
                                psts = [pst[:, 0:ln], pst[:, ln : 2 * ln]]
                            else:
                                if ci == 0:
                                    pa = pp.tile([P, 512], FP32, tag="fc1_ps")
                                    pb = pp.tile([P, 512], FP32, tag="fc1_ps")
                                else:
                                    pa = ptl.tile([P, 128], FP32, tag="fc1_tl")
                                    pb = ptl.tile([P, 128], FP32, tag="fc1_tl")
                                psts = [pa[:, 0:ln], pb[:, 0:ln]]
                            for hin in range(2):
                                h = 2 * q + hin
                                for d in range(DC):
                                    nc.tensor.matmul(
                                        psts[hin],
                                        lhsT=w1sb[:, h * DC * P + d * P : h * DC * P + (d + 1) * P],
                                        rhs=xsb[:, xoff[s] + d * cap + n0 : xoff[s] + d * cap + n1],
                                        start=(d == 0),
                                        stop=(d == DC - 1),
                                    )
                            if use_b1 or not paired:
                                for hin in range(2):
                                    h = 2 * q + hin
                                    nc.scalar.activation(
                                        hpark[:, hoff[s] + h * cap + n0 : hoff[s] + h * cap + n1],
                                        psts[hin],
                                        gelu_af,
                                        bias=b1sb[:, s * HC + h : s * HC + h + 1],
                                    )
                            else:
                                outv = hpark[
                                    :, hoff[s] + 2 * q * cap : hoff[s] + (2 * q + 2) * cap
                                ].rearrange("p (k n) -> p k n", k=2)[:, :, n0:n1]
                                inv = pst[:, 0 : 2 * ln].rearrange(
                                    "p (k n) -> p k n", k=2
                                )
                                nc.scalar.activation(outv, inv, gelu_af)

                # ---------- top-2 softmax gates (deferred: its Exp lands in
                # the same table-set region as the fc2 exps) ----------------
                W(0.085)
                g1sb = gatep.tile([P, NTT], FP32, tag="g1")
                g2sb = gatep.tile([P, NTT], FP32, tag="g2")
                trp_t = ppg.tile([P, 512], FP32, tag="gate_ps")
                trp = trp_t[:, 0 : NTT * E]
                for t in range(NTT):
                    nc.tensor.transpose(
                        trp[:, t * E : (t + 1) * E],
                        lg_sb[:, t * P : (t + 1) * P],
                        ident[:E, :E],
                    )
                lt8 = onp.tile([P, NTT * E], FP32, tag="lt8")
                nc.vector.tensor_copy(lt8[:], trp[:])
                mx8 = onp.tile([P, NTT * 8], FP32, tag="mx8")
                d21a = onp.tile([P, NTT], FP32, tag="d21a")
                e21a = onp.tile([P, NTT], FP32, tag="e21a")
                t1ga = onp.tile([P, NTT], FP32, tag="t1ga")
                for t in range(NTT):
                    nc.vector.max(
                        out=mx8[:, t * 8 : (t + 1) * 8],
                        in_=lt8[:, t * E : (t + 1) * E],
                    )
                    nc.vector.tensor_sub(
                        d21a[:, t : t + 1],
                        mx8[:, t * 8 + 1 : t * 8 + 2],
                        mx8[:, t * 8 : t * 8 + 1],
                    )
                nc.scalar.activation(e21a[:], d21a[:], AF.Exp)
                nc.vector.tensor_scalar_add(t1ga[:], e21a[:], 1.0)
                nc.vector.reciprocal(g1sb[:], t1ga[:])
                nc.vector.tensor_mul(g2sb[:], g1sb[:], e21a[:])

                # ---------- fc2 + exp + pipelined combines ----------
                a_dram = dp.tile([R, D], BF16, tag="a_tab")
                comb = gatep.tile([P, NTT * D], FP32, tag="comb")

                def emit_combine(t, pref, perm_from=None):
                    # perm_from = (asb_tile, m, perm_idx): rows of the final
                    # atile come via a PE permutation-matmul from SBUF; the
                    # gather is bounded to earlier rows (OOB rows skipped
                    # into the pre-zeroed buffer)
                    b12 = sp.tile([P, 2 * D], BF16, tag="b12")
                    if perm_from is not None:
                        nc.vector.memset(b12[:], 0.0)
                    if fused_gather:
                        nc.gpsimd.indirect_dma_start(
                            out=b12[:],
                            out_offset=None,
                            in_=a_dram[0:pref, :],
                            in_offset=bass.IndirectOffsetOnAxis(
                                ap=j12sb[:, 2 * t : 2 * t + 2], axis=0
                            ),
                        )
                    else:
                        bc = (pref - 1) if perm_from is not None else None
                        nc.gpsimd.indirect_dma_start(
                            out=b12[:, 0:D],
                            out_offset=None,
                            in_=a_dram[0:pref, :],
                            in_offset=bass.IndirectOffsetOnAxis(
                                ap=j12sb[:, 2 * t : 2 * t + 1], axis=0
                            ),
                            bounds_check=bc,
                            oob_is_err=(bc is None),
                        )
                        nc.gpsimd.indirect_dma_start(
                            out=b12[:, D : 2 * D],
                            out_offset=None,
                            in_=a_dram[0:pref, :],
                            in_offset=bass.IndirectOffsetOnAxis(
                                ap=j12sb[:, 2 * t + 1 : 2 * t + 2], axis=0
                            ),
                            bounds_check=bc,
                            oob_is_err=(bc is None),
                        )
                    AL = mybir.AluOpType
                    ct = comb[:, t * D : (t + 1) * D]
                    # gathered part first (off the late critical chain):
                    # ct = g1*b12_1 + g2*b12_2 in two fused DVE ops
                    nc.vector.tensor_scalar_mul(ct, b12[:, 0:D],
                                                g1sb[:, t : t + 1])
                    nc.vector.scalar_tensor_tensor(
                        ct, b12[:, D : 2 * D], g2sb[:, t : t + 1], ct,
                        AL.mult, AL.add,
                    )
                    if perm_from is not None:
                        pasb, pm, pidx = perm_from
                        ppa = pp2.tile([P, D], FP32, tag="perm_ps")
                        ppb = pp2.tile([P, D], FP32, tag="perm_ps")
                        for k, ppk in enumerate((ppa, ppb)):
                            nc.tensor.matmul(
                                ppk[:],
                                lhsT=pmsb[0:pm, (2 * pidx + k) * P : (2 * pidx + k + 1) * P],
                                rhs=pasb[0:pm, :],
                                start=True,
                                stop=True,
                            )
                        nc.vector.scalar_tensor_tensor(
                            ct, ppa[:], g1sb[:, t : t + 1], ct,
                            AL.mult, AL.add,
                        )
                        nc.vector.scalar_tensor_tensor(
                            ct, ppb[:], g2sb[:, t : t + 1], ct,
                            AL.mult, AL.add,
                        )

                pending_perm = []
                for g, (s, tt, m, pref) in enumerate(atiles):
                    W(0.09 + 0.003 * g)
                    flush_perm = pending_perm
                    pending_perm = []
                    cap = caps[s]
                    w2sb = w2_tiles[s]
                    ps2 = pp2.tile([P, D], FP32, tag="fc2_ps")
                    for h in range(HC):
                        nc.tensor.matmul(
                            ps2[:m],
                            lhsT=hpark[:, hoff[s] + h * cap + tt * P : hoff[s] + h * cap + tt * P + m],
                            rhs=w2sb[:, h * D : (h + 1) * D],
                            start=(h == 0),
                            stop=(h == HC - 1 and not use_b2),
                        )
                    if use_b2:
                        nc.tensor.matmul(
                            ps2[:m],
                            lhsT=ones1[:, :m],
                            rhs=b2sb[s : s + 1, :],
                            start=False,
                            stop=True,
                        )
                    asb = sp.tile([P, D], BF16, tag="a_sb")
                    nc.scalar.activation(asb[:m], ps2[:m], AF.Exp)
                    # the store is only needed if some gather reads this atile:
                    # unpermuted combines read atiles <= their rtile; permuted
                    # ones read atiles < their rtile
                    store_needed = any(
                        (rtiles[t] >= g and t not in perm_tiles)
                        or (rtiles[t] > g and t in perm_tiles)
                        for t in range(NTT)
                    )
                    if store_needed:
                        nc.sync.dma_start(
                            out=a_dram[offs[s] + tt * P : offs[s] + tt * P + m, :],
                            in_=asb[:m],
                        )
                    for t in range(NTT):
                        if rtiles[t] == g:
                            if t in perm_tiles:
                                pidx = perm_tiles.index(t)
                                prev_pref = atiles[g - 1][3] if g > 0 else 0
                                pending_perm.append(
                                    (t, prev_pref, (asb, m, pidx))
                                )
                            else:
                                emit_combine(t, pref)
                    for (pt, ppref, pfrom) in flush_perm:
                        emit_combine(pt, ppref, perm_from=pfrom)

                # ---------- final Ln (in place) + y stores ----------
                W(0.15)
                for (pt, ppref, pfrom) in pending_perm:
                    emit_combine(pt, ppref, perm_from=pfrom)
                W(0.3)
                for t in range(NTT):
                    nc.scalar.activation(
                        comb[:, t * D : (t + 1) * D],
                        comb[:, t * D : (t + 1) * D],
                        AF.Ln,
                    )
                    nc.sync.dma_start(
                        out=y_d[t * P : (t + 1) * P, :],
                        in_=comb[:, t * D : (t + 1) * D],
                    )
                tc.tile_set_cur_wait(0, enable=False)

            if reps > 1:
                with tc.For_i(0, reps, 1):
                    body()
            else:
                body()
            if timing:
                nc.sync.dma_start(out=yo_d[:], in_=ident[:1, :4])

    nc.compile()
    return nc


def _route(gate_feat, noise, w_gate, w_noise):
    """Host-side routing structure (fp32 numpy, matches jax top-k selection)."""
    clean = gate_feat @ w_gate
    stddev = np.logaddexp(gate_feat @ w_noise, 0.0) + np.float32(1e-2)
    logits = clean.astype(np.float32) + noise * stddev.astype(np.float32)
    top2 = np.argsort(-logits, axis=1, kind="stable")[:, :TOPK].astype(np.int32)
    return top2


def _pair_cands(pid_vals):
    bsets = [frozenset(b) for b in BLOCKS]
    cands = {}
    for p in pid_vals:
        i, j = int(p) // E, int(p) % E
        cs = [c for c, b in enumerate(bsets) if i in b and j in b]
        assert cs, f"pair ({i},{j}) not covered by block design"
        cands[int(p)] = cs
    return cands


def _assign_lp(pair, pid):
    """Pair-level LP: minimize total per-(core,expert) overflow beyond 512
    (ragged fc2 tiles), subject to exact pair counts and NS tokens/core."""
    from scipy.optimize import linprog

    upids = [int(p) for p in np.unique(pid)]
    n_p = {p: int((pid == p).sum()) for p in upids}
    cands = _pair_cands(upids)
    xv = [(p, c) for p in upids for c in cands[p]]
    xi = {k: i for i, k in enumerate(xv)}
    ov = [(c, e) for c in range(NC) for e in BLOCKS[c]]
    oi = {k: len(xv) + i for i, k in enumerate(ov)}
    nvar = len(xv) + len(ov)

    Aeq, beq = [], []
    for p in upids:
        row = np.zeros(nvar)
        for c in cands[p]:
            row[xi[(p, c)]] = 1
        Aeq.append(row)
        beq.append(n_p[p])
    for c in range(NC):
        row = np.zeros(nvar)
        for p in upids:
            if c in cands[p]:
                row[xi[(p, c)]] = 1
        Aeq.append(row)
        beq.append(NS)
    Aub, bub = [], []
    for (c, e) in ov:
        row = np.zeros(nvar)
        for p in upids:
            if c in cands[p] and (p // E == e or p % E == e):
                row[xi[(p, c)]] = 1
        row[oi[(c, e)]] = -1
        Aub.append(row)
        bub.append(512)
    cvec = np.zeros(nvar)
    cvec[len(xv):] = 1.0
    res = linprog(cvec, A_ub=np.array(Aub), b_ub=np.array(bub),
                  A_eq=np.array(Aeq), b_eq=np.array(beq),
                  bounds=[(0, None)] * nvar, method="highs")
    if res.status != 0:
        return None
    x = res.x[: len(xv)]
    xf = {k: int(np.floor(x[i] + 1e-9)) for k, i in xi.items()}
    for p in upids:
        rem = n_p[p] - sum(xf[(p, c)] for c in cands[p])
        fr = sorted(cands[p], key=lambda c: -(x[xi[(p, c)]] - xf[(p, c)]))
        for c in fr[:rem]:
            xf[(p, c)] += 1
    load = np.zeros(NC, dtype=np.int64)
    for (p, c), v in xf.items():
        load[c] += v
    # repair core totals (rounding drift) by shifting along shared pairs
    for _ in range(1000):
        if (load == NS).all():
            break
        moved = False
        for p in upids:
            cs = cands[p]
            for a in cs:
                if load[a] > NS and xf[(p, a)] > 0:
                    for b in cs:
                        if load[b] < NS:
                            xf[(p, a)] -= 1
                            xf[(p, b)] += 1
                            load[a] -= 1
                            load[b] += 1
                            moved = True
                            break
                if moved:
                    break
            if moved:
                break
        if not moved:
            return None
    if not (load == NS).all():
        return None
    return xf


def _assign_greedy(pair, pid):
    """Fallback greedy (load-balanced, per-expert max minimized)."""
    cands = _pair_cands([int(p) for p in np.unique(pid)])
    ncand = np.array([len(cands[int(p)]) for p in pid])
    prio = np.where(ncand == 1, 0, np.where(ncand == 3, 1, 2))
    order = np.argsort(prio, kind="stable")
    core_of_tok = np.full(N, -1, dtype=np.int64)
    load = np.zeros(NC, dtype=np.int64)
    eload = np.zeros((NC, E), dtype=np.int64)
    for t in order:
        cs = cands[int(pid[t])]
        i, j = int(pair[t, 0]), int(pair[t, 1])
        avail = [c for c in cs if load[c] < NS]
        if not avail:
            moved = False
            for c in cs:
                for u in np.where(core_of_tok == c)[0]:
                    for c2 in cands[int(pid[u])]:
                        if c2 != c and load[c2] < NS:
                            core_of_tok[u] = c2
                            load[c] -= 1
                            load[c2] += 1
                            ui, uj = int(pair[u, 0]), int(pair[u, 1])
                            eload[c, ui] -= 1
                            eload[c, uj] -= 1
                            eload[c2, ui] += 1
                            eload[c2, uj] += 1
                            moved = True
                            break
                    if moved:
                        break
                if moved:
                    avail = [c]
                    break
            assert moved, "assignment infeasible"
        best = min(avail, key=lambda c: (load[c], max(eload[c, i], eload[c, j])))
        core_of_tok[t] = best
        load[best] += 1
        eload[best, i] += 1
        eload[best, j] += 1
    assert (load == NS).all()
    return core_of_tok, eload


def _assign_cores(top2):
    """Token -> core assignment: each token to a core whose 4-expert block
    contains both of its experts; exactly NS tokens per core; per-(core,
    expert) loads pushed toward <=512 to minimize ragged fc2 tiles."""
    pair = np.sort(top2, axis=1)
    pid = pair[:, 0] * E + pair[:, 1]
    xf = None
    try:
        xf = _assign_lp(pair, pid)
    except Exception:
        xf = None
    if xf is None:
        return _assign_greedy(pair, pid)
    core_of_tok = np.full(N, -1, dtype=np.int64)
    eload = np.zeros((NC, E), dtype=np.int64)
    for p in np.unique(pid):
        toks = np.where(pid == p)[0]
        pos = 0
        i, j = int(p) // E, int(p) % E
        for c in _pair_cands([int(p)])[int(p)]:
            v = xf.get((int(p), c), 0)
            for t in toks[pos : pos + v]:
                core_of_tok[t] = c
            eload[c, i] += v
            eload[c, j] += v
            pos += v
        assert pos == len(toks)
    assert (core_of_tok >= 0).all()
    return core_of_tok, eload


def _prepare(x, gate_feat, noise, w_gate, w_noise, fc1_w, fc1_b, fc2_w, fc2_b):
    x = np.ascontiguousarray(x, dtype=np.float32)
    gate_feat = np.ascontiguousarray(gate_feat, dtype=np.float32)
    noise = np.ascontiguousarray(noise, dtype=np.float32)
    bf = ml_dtypes.bfloat16

    top2 = _route(gate_feat, noise, w_gate, w_noise)
    core_of_tok, eload = _assign_cores(top2)
    pair = np.sort(top2, axis=1)

    # segment order per core: hosted experts by descending count
    seg_expert = np.zeros((NC, SEG), dtype=np.int64)
    for c in range(NC):
        hosted = np.array(BLOCKS[c])
        cnts = eload[c, hosted]
        seg_expert[c] = hosted[np.argsort(-cnts, kind="stable")]
    seg_cnts = np.stack(
        [[eload[c, seg_expert[c, s]] for s in range(SEG)] for c in range(NC)]
    )
    caps = seg_cnts.max(axis=0)
    offs = np.concatenate([[0], np.cumsum(caps)]).astype(np.int64)
    R = int(offs[-1])

    # a-store tile index of each global row
    atiles = _atiles(caps)
    atile_of_row = np.zeros(R, dtype=np.int64)
    for g, (s, tt, m, pref) in enumerate(atiles):
        r0 = int(offs[s]) + tt * P
        atile_of_row[r0 : r0 + m] = g

    wg_bf = np.ascontiguousarray(w_gate).astype(bf)
    wn_bf = np.ascontiguousarray(w_noise).astype(bf)
    wgp = np.ascontiguousarray(
        wg_bf.reshape(FC, P, E).transpose(1, 0, 2).reshape(P, FC * E)
    )
    wnp = np.ascontiguousarray(
        wn_bf.reshape(FC, P, E).transpose(1, 0, 2).reshape(P, FC * E)
    )

    in_maps = []
    gidx_cores = []
    rtiles_cores = []
    jp_cores = []
    for c in range(NC):
        toks = np.where(core_of_tok == c)[0]        # 1024 global ids, ascending
        t2 = pair[toks]                              # sorted pairs
        seg_of = np.full(E, -1, dtype=np.int64)
        for s in range(SEG):
            seg_of[seg_expert[c, s]] = s

        row_of = {}                                  # (local_t, expert) -> row
        cols = np.zeros(R, dtype=np.int64)           # row -> local token
        for s in range(SEG):
            e = seg_expert[c, s]
            lt = np.where((t2[:, 0] == e) | (t2[:, 1] == e))[0]
            for i, l in enumerate(lt):
                row_of[(int(l), int(e))] = int(offs[s]) + i
            cols[offs[s] : offs[s] + len(lt)] = lt

        e1 = top2[toks, 0]
        e2 = top2[toks, 1]
        j1 = np.array([row_of[(l, int(e1[l]))] for l in range(NS)], dtype=np.int32)
        j2 = np.array([row_of[(l, int(e2[l]))] for l in range(NS)], dtype=np.int32)
        ready = np.maximum(atile_of_row[j1], atile_of_row[j2])
        perm = np.argsort(ready, kind="stable")
        rtiles_cores.append(ready[perm].reshape(NTT, P).max(axis=1))

        # xt [P, DC*R] segment-major: [p, xoff[s] + d*cap_s + i]
        x_loc = x[toks]                              # [NS, D]
        xt = np.zeros((P, DC * R), dtype=bf)
        for s in range(SEG):
            cap = int(caps[s])
            cnt = int(seg_cnts[c, s])
            xs = x_loc[cols[offs[s] : offs[s] + cnt]]         # [cnt, D]
            blk = xs.reshape(cnt, DC, P).transpose(2, 1, 0)   # [P, DC, cnt]
            base = int(4 * offs[s])
            for d in range(DC):
                xt[:, base + d * cap : base + d * cap + cnt] = blk[:, d, :]

        gf_loc = gate_feat[toks][perm]               # [NS, 2D]
        gftp = np.ascontiguousarray(
            gf_loc.reshape(2, 512, FC, P).transpose(3, 0, 2, 1).reshape(P, 2 * FC * 512)
        ).astype(bf)
        ns_loc = noise[toks][perm]

        w1p_ = np.zeros((SEG, P, HC * DC * P), dtype=bf)
        w2p_ = np.zeros((SEG, P, HC * D), dtype=bf)
        b1p_ = np.zeros((P, SEG * HC), dtype=np.float32)
        b2p_ = np.zeros((SEG, D), dtype=bf)
        for s in range(SEG):
            e = int(seg_expert[c, s])
            # w1: [p, hc*512 + d*128 + j] = fc1_w[e, hc*128+j, d*128+p]
            w1p_[s] = (
                np.asarray(fc1_w[e]).reshape(HC, P, DC, P)
                .transpose(3, 0, 2, 1).reshape(P, HC * DC * P).astype(bf)
            )
            # w2: [p, hc*D + dd] = fc2_w[e, dd, hc*128+p]
            w2p_[s] = (
                np.asarray(fc2_w[e]).T.reshape(HC, P, D)
                .transpose(1, 0, 2).reshape(P, HC * D).astype(bf)
            )
            b1p_[:, s * HC : (s + 1) * HC] = np.asarray(fc1_b[e]).reshape(HC, P).T
            b2p_[s] = np.asarray(fc2_b[e]).astype(bf)

        # permutation blocks for combine tiles resolved via PE permute
        atl = _atiles(caps)
        g3start = next(g for g, (s_, tt_, m_, pref_) in enumerate(atl)
                       if s_ == SEG - 1)
        # NOTE: perm_tiles must match the device build (computed from the
        # global rtiles); filled in after the cross-core max — see below.
        in_maps.append({
            "xt": np.ascontiguousarray(xt),
            "gft": gftp,
            "nst": np.ascontiguousarray(ns_loc.T.astype(np.float32)),
            "wg": wgp,
            "wn": wnp,
            "w1": np.ascontiguousarray(w1p_),
            "w2": np.ascontiguousarray(w2p_),
            "b1": b1p_,
            "b2": b2p_,
            "j12": np.ascontiguousarray(
                np.stack([j1[perm].reshape(NTT, P).T,
                          j2[perm].reshape(NTT, P).T], axis=2).reshape(P, 2 * NTT)
            ),
        })
        gidx_cores.append(toks[perm])
        jp = np.stack([j1[perm].reshape(NTT, P).T,
                       j2[perm].reshape(NTT, P).T], axis=2)  # [P, NTT, 2]
        jp_cores.append(jp)

    rtiles = tuple(int(v) for v in np.max(np.stack(rtiles_cores), axis=0))

    # permutation blocks: tiles whose (global) rtile falls in the last
    # segment take their final-atile rows via a PE permute from SBUF
    atl = _atiles(caps)
    g3start = next(g for g, (s_, tt_, m_, pref_) in enumerate(atl)
                   if s_ == SEG - 1)
    perm_tiles = [t for t in range(NTT) if rtiles[t] >= g3start]
    row_base = {g: (int(np.concatenate([[0], np.cumsum(caps)])[s_]) + tt_ * P)
                for g, (s_, tt_, m_, pref_) in enumerate(atl)}
    atile_of = atile_of_row
    for c in range(NC):
        pmt = np.zeros((P, max(1, len(perm_tiles)) * 2 * P), dtype=ml_dtypes.bfloat16)
        jp = jp_cores[c]
        for pidx, t in enumerate(perm_tiles):
            g = rtiles[t]
            base = row_base[g]
            for k in range(2):
                rows = jp[:, t, k]          # [P] row of token (t, p), side k
                inat = atile_of[rows] == g
                for p in np.where(inat)[0]:
                    pmt[rows[p] - base, (2 * pidx + k) * P + p] = 1
        in_maps[c]["pmt"] = np.ascontiguousarray(pmt)
    return tuple(int(v) for v in caps), rtiles, gidx_cores, in_maps


def kernel(x, gate_feat, noise, w_gate, w_noise, fc1_w, fc1_b, fc2_w, fc2_b,
           _reps=1):
    caps, rtiles, gidx_cores, in_maps = _prepare(
        x, gate_feat, noise, w_gate, w_noise, fc1_w, fc1_b, fc2_w, fc2_b
    )
    use_b2 = bool(np.any(np.asarray(fc2_b)))
    use_b1 = bool(np.any(np.asarray(fc1_b)))
    key = (caps, rtiles, int(_reps), use_b2, use_b1)
    if key not in _nc_cache:
        _nc_cache[key] = _build_nc(caps, rtiles, reps=_reps, use_b2=use_b2,
                                   use_b1=use_b1)
    nc = _nc_cache[key]
    try:
        res = run_bass_kernel_spmd(nc, in_maps, core_ids=list(range(NC)))
    except Exception:
        res = run_bass_kernel_spmd(nc, in_maps, core_ids=list(range(NC)))
    y = np.empty((N, D), np.float32)
    for c in range(NC):
        y[gidx_cores[c]] = res.results[c]["y"]
    return y
